# revision 81
# baseline (speedup 1.0000x reference)
"""Multi-head attention (B=4, S=2048, D=1024, H=16) on 8 NeuronCores.

Sharding: core c -> (batch b = c//2, head-group g = c%2 of 8 heads).
Each core computes QKV projections for its 8 heads, causal attention, and a
row-sharded output projection partial; the host sums the two partials per
batch and adds the output bias.

Device kernel layout choices:
  * QKV projections run as fp8-e4m3 DoubleRow matmuls (0.5 cycles/row over
    a 256-deep contraction) with hi+lo error compensation on both operands:
    x*w ~ xh*wh + xl*wh + xh*wl, matching bf16 accuracy at 0.75x the PE
    cost. Weights are pre-scaled by 32 on the host so their magnitudes sit
    in e4m3's normal range; the scale is undone in the exp argument (q.k)
    and the normalization multiply (v). Host ships hi/lo pre-split, pair-
    chunked fp8 tensors laid out so each chunk loads with ONE 3-dim DMA.
  * Q/K are produced in transposed layout (head-dim on partitions) so the
    score matmuls ST = K @ Q^T contract over d_k on the partition axis;
    per-partition Q/K biases ride the ScalarE PSUM->SBUF evacuation free.
  * Scores stay transposed (k on partitions, q on free dim): softmax needs
    no partition reductions. exp runs on ScalarE over the causally-live
    window only, with a fixed -3 bias (cancels in normalization, keeps
    bf16 P in a comfortable range); the Exp table loads exactly once (no
    ln/reciprocal on ScalarE).
  * AV matmuls are PE-cheap by making P the STATIONARY operand:
    out[128 q, 64 d + den] accumulates per (k-block, q-sub, head) with the
    65-wide [V | 1] moving operand -- 65 cycles instead of 512 per block-
    head. The ones column accumulates the softmax denominator in the same
    PSUM tile.
  * Normalization = DVE reciprocal of the per-token denominator column +
    one fused (x * rec) * (1/32) tensor-scalar per unit; the [q, d] ->
    [d, q] flip to the output-projection layout rides the DMA transpose
    crossbar (one 128x128 tile per q-sub, both heads at once). No PE
    broadcast matmuls.
  * The boolean mask is classified on the host at (512 q x 128 k) block
    granularity into skip / full / partial-with-pattern; patterns are
    deduplicated (causal tril -> a single 128x512 tile) and applied as
    multiplicative 0/1 masks after exp. Dead q-columns inside 128-aligned
    sub-blocks are zeroed so they can feed stationary P tiles.
  * Emission is software-pipelined: each block's AV matmuls are deferred
    one block behind its scores/exp, and ~1us quanta of projection /
    output-projection work are interleaved as PE filler (r-outer loop so
    range r's output projection drains during range r+1).
"""

import numpy as np
import ml_dtypes
from contextlib import ExitStack

import concourse.bass as bass
import concourse.bacc as bacc
import concourse.tile as tile
from concourse import mybir
from concourse.bass_utils import run_bass_kernel_spmd

F32 = mybir.dt.float32
BF16 = mybir.dt.bfloat16
FP8 = mybir.dt.float8e4
BF = ml_dtypes.bfloat16
E4 = ml_dtypes.float8_e4m3

B, S, D, H, DK = 4, 2048, 1024, 16, 64
NCORES = 8
GH = 8            # heads per core
DL = GH * DK      # 512 local feature dims
NPAIR = 4         # local head pairs
NR = 4            # q ranges of 512
NKB = S // 128    # 16 k blocks
NC2 = D // 256    # 4 paired contraction chunks
EXP = mybir.ActivationFunctionType.Exp
IDENT = mybir.ActivationFunctionType.Identity
DR = mybir.MatmulPerfMode.DoubleRow
WSCALE = 32.0     # host-side weight scale (fp8 subnormal avoidance)
SCALE = 1.0 / np.sqrt(DK)
EXPBIAS = -3.0    # keeps exp outputs in a comfy range; cancels in softmax


class BlockInfo:
    __slots__ = ("j", "live0", "live1", "pat", "mul0", "mul1")

    def __init__(self, j, live0, live1, pat, mul0, mul1):
        self.j, self.live0, self.live1 = j, live0, live1
        self.pat, self.mul0, self.mul1 = pat, mul0, mul1


def classify_mask(mask):
    """Classify (512 q x 128 k) blocks of the attention mask.

    Returns (live, patterns): live[r] is a list of BlockInfo for the k-blocks
    that have any attendable position; patterns is a list of (128, 512)
    float32 0/1 tiles (k on rows, q-local on cols), deduplicated.
    """
    live = []
    patterns = []
    index = {}
    for r in range(NR):
        row = []
        qs = mask[512 * r: 512 * (r + 1), :]
        for j in range(NKB):
            blk = qs[:, 128 * j: 128 * (j + 1)]       # (512 q, 128 k)
            if not blk.any():
                continue
            if blk.all():
                row.append(BlockInfo(j, 0, 512, None, 0, 0))
                continue
            bt = blk.T                                  # (128 k, 512 q)
            colfull = bt.all(axis=0)
            colany = bt.any(axis=0)
            liveidx = np.nonzero(colany)[0]
            live0, live1 = int(liveidx.min()), int(liveidx.max()) + 1
            nonfull = np.nonzero(~colfull[live0:live1])[0]
            if len(nonfull) == 0:
                row.append(BlockInfo(j, live0, live1, None, 0, 0))
                continue
            mul0 = live0 + int(nonfull.min())
            mul1 = live0 + int(nonfull.max()) + 1
            pat = bt[:, mul0:mul1].astype(np.float32)
            key = (mul1 - mul0, pat.tobytes())
            if key not in index:
                index[key] = len(patterns)
                padded = np.zeros((128, 512), np.float32)
                padded[:, : mul1 - mul0] = pat
                patterns.append(padded)
            row.append(BlockInfo(j, live0, live1, index[key], mul0, mul1))
        if not row:
            raise NotImplementedError(
                "a 512-row q range attends to nothing; fully-masked rows "
                "are not supported"
            )
        live.append(row)
    if len(patterns) > 8:
        raise NotImplementedError(f"{len(patterns)} unique mask patterns")
    return live, patterns


def build_program(live, n_pat):
    nc = bacc.Bacc("TRN2", target_bir_lowering=False, debug=False,
                   num_devices=NCORES)

    d = {}
    # (chunk, partition, hi/lo, pair, col): chunk-sliced DMAs collapse to
    # 3 access-pattern dims
    for nm in ("xq", "xk", "xv"):
        d[nm + "8"] = nc.dram_tensor(
            nm + "8", [NC2, 128, 2, 2, S], FP8, kind="ExternalInput").ap()
    for nm in ("wq", "wk", "wv"):
        d[nm + "8"] = nc.dram_tensor(
            nm + "8", [NC2, 128, 2, 2, DL], FP8, kind="ExternalInput").ap()
    wot = nc.dram_tensor("wot", [DL, D], BF16, kind="ExternalInput").ap()
    bqkt = nc.dram_tensor("bqkt", [128, 2 * NPAIR], F32,
                          kind="ExternalInput").ap()
    bvd = nc.dram_tensor("bv", [1, DL], BF16, kind="ExternalInput").ap()
    patd = nc.dram_tensor("pats", [max(n_pat, 1), 128, 512], BF16,
                          kind="ExternalInput").ap()
    outp = nc.dram_tensor("outp", [S, D], F32, kind="ExternalOutput").ap()

    with tile.TileContext(nc) as tc, ExitStack() as ctx:
        emit(ctx, tc, nc, live, n_pat, d, wot, bqkt, bvd, patd, outp)
    nc.compile()
    return nc


def emit(ctx, tc, nc, live, n_pat, d, wot, bqkt, bvd, patd, outp):
    wpool = ctx.enter_context(tc.tile_pool(name="w", bufs=1))
    qkpool = ctx.enter_context(tc.tile_pool(name="qk", bufs=1))
    vpool = ctx.enter_context(tc.tile_pool(name="vp", bufs=1))
    otpool = ctx.enter_context(tc.tile_pool(name="otp", bufs=1))
    xs = ctx.enter_context(tc.tile_pool(name="xs", bufs=4))
    ptp = ctx.enter_context(tc.tile_pool(name="ptp", bufs=2))
    nrm = ctx.enter_context(tc.tile_pool(name="nrm", bufs=8))
    outs = ctx.enter_context(tc.tile_pool(name="outs", bufs=2))

    pps = ctx.enter_context(tc.tile_pool(name="pps", bufs=2, space="PSUM"))
    stps = ctx.enter_context(tc.tile_pool(name="stps", bufs=2, space="PSUM"))
    avps = ctx.enter_context(tc.tile_pool(name="avps", bufs=1, space="PSUM"))
    ops = pps

    # ---- resident tiles ----
    bv_sb = wpool.tile([1, DL], BF16, tag="bv", name="bv")
    nc.sync.dma_start(bv_sb[:], bvd)
    # per-partition q/k biases: col hp = bq[128hp:128hp+128], col 4+hp = bk
    bqk_sb = wpool.tile([128, 2 * NPAIR], F32, tag="bqk", name="bqk")
    nc.sync.dma_start(bqk_sb[:], bqkt)
    pat_sb = []
    for i in range(n_pat):
        t = wpool.tile([128, 512], BF16, tag=f"pat{i}", name=f"pat{i}")
        nc.sync.dma_start(t[:], patd[i])
        pat_sb.append(t)

    def alloc8(name, free):
        # (partition, hi/lo, pair, col): one DMA loads hi+lo of a chunk
        return wpool.tile([128, 2, 2, free], FP8, tag=name, name=name)

    # paired-contraction fp8 tiles: element (p, a, i, n) has hi/lo part a of
    # matrix row 256c + 128i + p
    xq8_t = [alloc8(f"xq{c}", S) for c in range(NC2)]
    xk8_t = [alloc8(f"xk{c}", S) for c in range(NC2)]
    wq8_t = [alloc8(f"wq{c}", DL) for c in range(NC2)]
    wk8_t = [alloc8(f"wk{c}", DL) for c in range(NC2)]
    wv8_t = [alloc8(f"wv{c}", DL) for c in range(NC2)]
    wo_t = [wpool.tile([128, 512], BF16, tag=f"wo{i}", name=f"wo{i}")
            for i in range(2 * NPAIR)]

    AIDX = {"h": 0, "l": 1}

    def pair_rows(dram, c, cols):
        return dram[c][:, :, :, cols]

    def load_w_chunk(hp):
        cols = slice(128 * hp, 128 * (hp + 1))
        for c in range(NC2):
            nc.gpsimd.dma_start(wq8_t[c][:, :, :, cols],
                                pair_rows(d["wq8"], c, cols))
            nc.gpsimd.dma_start(wk8_t[c][:, :, :, cols],
                                pair_rows(d["wk8"], c, cols))

    def load_x_chunk(sc):
        cols = slice(512 * sc, 512 * (sc + 1))
        for c in range(NC2):
            nc.sync.dma_start(xq8_t[c][:, :, :, cols],
                              pair_rows(d["xq8"], c, cols))
            nc.sync.dma_start(xk8_t[c][:, :, :, cols],
                              pair_rows(d["xk8"], c, cols))

    def load_wv():
        for c in range(NC2):
            nc.gpsimd.dma_start(wv8_t[c][:],
                                pair_rows(d["wv8"], c, slice(0, DL)))

    def load_wo():
        for i in range(2 * NPAIR):
            nc.gpsimd.dma_start(
                wo_t[i][:], wot[128 * (i // 2):128 * (i // 2 + 1),
                                512 * (i % 2):512 * (i % 2 + 1)])

    ones_bf = wpool.tile([1, 512], BF16, tag="ones_bf")
    nc.gpsimd.memset(ones_bf[:], 1.0)
    # per-partition exp bias column (const-AP database only has 0.0/1.0)
    expb = wpool.tile([128, 1], F32, tag="expb")
    nc.gpsimd.memset(expb[:], EXPBIAS)

    HILO = (("h", "h"), ("l", "h"), ("h", "l"))

    va_t = [vpool.tile([128, GH * 65], BF16, tag=f"va{t}", name=f"va{t}")
            for t in range(NKB)]
    ot_t = [otpool.tile([128, S], BF16, tag=f"ot{hp}", name=f"ot{hp}")
            for hp in range(NPAIR)]

    qt_t, kt_t = {}, {}

    # ---- Q/K projection for one head pair (transposed layout) ----
    # fp8 DoubleRow with hi/lo compensation; bias add is free: the
    # PSUM->SBUF evacuation runs on ScalarE as an Identity activation with
    # a per-partition bias vector (dims are on partitions in this layout)
    def qk_proj_sc(hp, sc, which="qk"):
        wcols = slice(128 * hp, 128 * (hp + 1))
        xcols = slice(512 * sc, 512 * (sc + 1))
        groups = {
            "q": (xq8_t, wq8_t, hp, lambda: qt_t[hp]),
            "k": (xk8_t, wk8_t, NPAIR + hp, lambda: kt_t[hp]),
        }
        for w in which:
            x_t, w_t, bcol, dest = groups[w]
            ps = pps.tile([128, 512], F32, tag="pps")
            n = 0
            for c in range(NC2):
                for a, b in HILO:
                    n += 1
                    nc.tensor.matmul(
                        ps[:], w_t[c][:, AIDX[b], :, wcols],
                        x_t[c][:, AIDX[a], :, xcols],
                        start=(n == 1), stop=(n == 3 * NC2), perf_mode=DR)
            # evacuate on DVE (per-partition bias add) so filler
            # projections never contend with exp on ScalarE
            nc.vector.tensor_scalar_add(
                dest()[:, xcols], ps[:], bqk_sb[:, bcol:bcol + 1])

    # ---- V projection (natural layout, ones-augmented) ----
    def v_load(t0, t1):
        assert 0 < t1 - t0 <= 4
        chunks = {}
        for c in range(NC2):
            xt = xs.tile([128, 2, 2, 512], FP8, tag="xv",
                         name=f"xv{t0}_{c}", bufs=8)
            nc.sync.dma_start(
                xt[:, :, :, 0:128 * (t1 - t0)],
                pair_rows(d["xv8"], c, slice(128 * t0, 128 * t1)))
            chunks[c] = xt
        return chunks

    def v_mm(t, chunks, t0):
        o = 128 * (t - t0)
        ps = pps.tile([128, 512], F32, tag="pps")
        n = 0
        for c in range(NC2):
            for a, b in HILO:
                n += 1
                nc.tensor.matmul(
                    ps[:], chunks[c][:, AIDX[a], :, o:o + 128],
                    wv8_t[c][:, AIDX[b], :, :],
                    start=(n == 1), stop=False, perf_mode=DR)
        nc.tensor.matmul(ps[:], ones_bf[0:1, 0:128], bv_sb[0:1, :],
                         start=False, stop=True)
        va = va_t[t].rearrange("p (h w) -> p h w", w=65)
        nc.vector.tensor_copy(
            va[:, :, 0:64], ps.rearrange("p (h w) -> p h w", w=64))
        nc.gpsimd.memset(va[:, :, 64:65], 1.0)

    # ---- attention for one head pair, one q-range ----
    # software-pipelined: each block's AV matmuls are deferred by one block
    # so the PE never head-of-line blocks on ScalarE's exp; pop_filler()
    # interleaves ~1us quanta of independent projection work.
    # AV runs transposed: stationary = P tile [128k, 128q], moving =
    # [V | 1] columns [128k, 65] -> out [128 q, 64 dims + den] in PSUM at 65
    # cycles per (block, q-sub, head) instead of 512 per (block, head).
    # Normalization is then a per-partition (per-token) scalar multiply on
    # DVE, and the [q, d] -> [d, q] layout flip rides the DMA transpose
    # crossbar -- no PE broadcast matmuls, no reciprocal DMA round-trip.
    def attention_r(hp, r, pop_filler, navail):
        qt, kt_ = qt_t[hp], kt_t[hp]
        js = live[r]
        nj = len(js)
        # spread filler pops across the whole block loop: the PE queue is
        # in-order, so front-loaded fillers can't hide the later blocks'
        # exp-wait; one pop every `stride` blocks matches supply to demand
        stride = max(2, nj // max(1, min(navail, nj)))
        # units u = 2*s + h for q-sub s, head h; unit u lives in
        # avq[u // 4] columns [65*(u % 4), 65*(u % 4) + 65)
        avq = [avps.tile([128, 512], F32, tag=f"avq{i}",
                         name=f"avq{r}_{i}") for i in range(2)]
        first = [True, True]

        def score_block(j, lo, hi, st_out):
            nc.tensor.matmul(
                st_out[:, lo:hi],
                kt_[0:64, 128 * j:128 * (j + 1)],
                qt[0:64, 512 * r + lo:512 * r + hi],
                start=True, stop=True, tile_position=(0, 0))
            nc.tensor.matmul(
                st_out[:, 512 + lo:512 + hi],
                kt_[64:128, 128 * j:128 * (j + 1)],
                qt[64:128, 512 * r + lo:512 * r + hi],
                start=True, stop=True, tile_position=(64, 0))

        def emit_av(bi, pt, s0_, s1_, last):
            for s in range(s0_, s1_):
                for h in range(2):
                    hl = 2 * hp + h
                    u = 2 * s + h
                    nc.tensor.matmul(
                        avq[u // 4][:, 65 * (u % 4):65 * (u % 4) + 65],
                        pt[:, 512 * h + 128 * s:512 * h + 128 * (s + 1)],
                        va_t[bi.j][:, 65 * hl:65 * (hl + 1)],
                        start=first[u // 4], stop=last,
                        skip_group_check=True)
                    first[u // 4] = False

        from collections import deque as _dq
        pend = _dq()
        for ji, bi in enumerate(js):
            lo, hi = bi.live0, bi.live1
            lo128 = lo & ~127
            hi128 = (hi + 127) & ~127
            st = stps.tile([128, 1024], F32, tag="st")
            score_block(bi.j, lo, hi, st)
            pt = ptp.tile([128, 1024], BF16, tag="pt", bufs=4)
            stv = st.rearrange("p (h w) -> p h w", h=2)[:, :, lo:hi]
            ptv = pt.rearrange("p (h w) -> p h w", h=2)[:, :, lo:hi]
            nc.scalar.activation(ptv, stv, EXP,
                                 scale=float(SCALE / WSCALE ** 2),
                                 bias=expb[:, 0:1])
            # dead q-columns inside the 128-aligned sub-block range must be
            # zero: they feed the stationary P tiles
            for h in range(2):
                s0c = 512 * h
                if lo128 < lo:
                    nc.vector.memset(pt[:, s0c + lo128:s0c + lo], 0.0)
                if hi < hi128:
                    nc.vector.memset(pt[:, s0c + hi:s0c + hi128], 0.0)
            if bi.pat is not None:
                for h in range(2):
                    s0c = 512 * h
                    sl = pt[:, s0c + bi.mul0:s0c + bi.mul1]
                    nc.vector.tensor_mul(
                        sl, sl, pat_sb[bi.pat][:, 0:bi.mul1 - bi.mul0])
            if len(pend) >= (2 if nj >= 8 else 1):
                if ji % stride == 1 % stride:
                    pop_filler()
                emit_av(*pend.popleft(), last=False)
            pend.append((bi, pt, lo128 // 128, hi128 // 128))
        while pend:
            emit_av(*pend.popleft(), last=(len(pend) == 0))

        # evacuate + normalize: reciprocal of the per-token denominator
        # column, then one fused (x * rec) * (1/WSCALE) DVE op per unit;
        # the DMA crossbar transposes [q, d] tiles into the ot layout
        den = nrm.tile([128, 8], F32, tag="den", name=f"den{hp}_{r}",
                       bufs=4)
        for i in range(2):
            nc.vector.tensor_copy(
                den[:, 4 * i:4 * (i + 1)],
                avq[i][:, 0:260].rearrange("p (u w) -> p u w",
                                           w=65)[:, :, 64])
        rec = nrm.tile([128, 8], F32, tag="rec", name=f"rec{hp}_{r}",
                       bufs=4)
        nc.vector.reciprocal(out=rec[:], in_=den[:])
        avn = nrm.tile([128, 512], BF16, tag="avn", name=f"avn{hp}_{r}",
                       bufs=3)
        ot = ot_t[hp]
        for u in range(8):
            nc.vector.tensor_scalar(
                out=avn[:, 64 * u:64 * (u + 1)],
                in0=avq[u // 4][:, 65 * (u % 4):65 * (u % 4) + 64],
                scalar1=rec[:, u:u + 1], scalar2=float(1.0 / WSCALE),
                op0=mybir.AluOpType.mult, op1=mybir.AluOpType.mult)
        for s in range(4):
            # one 128x128 crossbar transpose flips both heads of q-sub s
            # into the [dims, tokens] ot layout at once
            nc.sync.dma_start_transpose(
                ot[:, 512 * r + 128 * s:512 * r + 128 * (s + 1)],
                avn[:, 128 * s:128 * (s + 1)])

    # ---- output projection partials ----
    def o_proj_unit(t, nh, hps, dest):
        ps = ops.tile([128, 512], F32, tag="pps", name="ops_ps")
        for i, hp in enumerate(hps):
            nc.tensor.matmul(
                ps[:], ot_t[hp][:, 128 * t:128 * (t + 1)],
                wo_t[2 * hp + nh][:],
                start=(i == 0), stop=(i == len(hps) - 1))
        osb = outs.tile([128, 512], F32, tag="osb", bufs=4)
        nc.vector.tensor_copy(osb[:], ps[:])
        nc.sync.dma_start(
            dest[128 * t:128 * (t + 1), 512 * nh:512 * (nh + 1)], osb[:])

    from collections import deque
    fill = deque()    # (key, thunk): ~1us quanta of deferrable PE work

    def pop_filler():
        if fill:
            fill.popleft()[1]()

    def force(pred):
        keep = deque()
        while fill:
            key, th = fill.popleft()
            if pred(key):
                th()
            else:
                keep.append((key, th))
        fill.extend(keep)

    # per-r requirements for phased production (exploits mask sparsity):
    #   kt/qt chunks up to need_sc[r], V tiles up to need_vt[r]
    maxj = [max(bi.j for bi in live[r]) for r in range(NR)]
    need_sc = [max(r, (maxj[r] * 128) // 512) for r in range(NR)]
    need_vt = [maxj[r] + 1 for r in range(NR)]
    for r in range(1, NR):
        need_sc[r] = max(need_sc[r], need_sc[r - 1])
        need_vt[r] = max(need_vt[r], need_vt[r - 1])

    qk_pushed = [0] * NPAIR
    w_loaded = [False] * NPAIR
    state = {"x": 0, "vt": 0}

    def push_qk(hp, upto):
        if not w_loaded[hp]:
            load_w_chunk(hp)
            w_loaded[hp] = True
            qt_t[hp] = qkpool.tile([128, S], BF16, tag=f"qt{hp}",
                                   name=f"qt{hp}")
            kt_t[hp] = qkpool.tile([128, S], BF16, tag=f"kt{hp}",
                                   name=f"kt{hp}")
        while qk_pushed[hp] <= upto:
            sc = qk_pushed[hp]
            while state["x"] <= sc:
                load_x_chunk(state["x"])
                state["x"] += 1
            for w in "qk":
                fill.append((("qk", hp, sc, w),
                             lambda hp=hp, sc=sc, w=w: qk_proj_sc(hp, sc, w)))
            qk_pushed[hp] += 1

    def force_qk(hp, upto):
        push_qk(hp, upto)
        force(lambda k: k[0] == "qk" and k[1] == hp and k[2] <= upto)

    def push_v(upto):
        while state["vt"] < upto:
            t0 = state["vt"]
            t1 = min(t0 + 4, upto)
            chunks = v_load(t0, t1)
            for t in range(t0, t1):
                fill.append((("v", t),
                             lambda t=t, c=chunks, o=t0: v_mm(t, c, o)))
            state["vt"] = t1

    def force_v(upto):
        force(lambda k: k[0] == "v" and k[1] < upto)

    push_qk(0, need_sc[0])
    force_qk(0, need_sc[0])
    # r-outer order: range r's output projection becomes available right
    # after head-pair 3's attention and is consumed as filler during range
    # r+1 -- no big serial o-proj tail
    for r in range(NR):
        for hp in range(NPAIR):
            if hp == 0:
                if r == 0:
                    load_wv()
                push_v(need_vt[r])
                force_v(need_vt[r])
                if r == 1:
                    load_wo()
            force_qk(hp, need_sc[r])
            # stage the next attention's projections as poppable filler
            if hp + 1 < NPAIR:
                push_qk(hp + 1, need_sc[r])
            elif r + 1 < NR:
                push_qk(0, need_sc[r + 1])
                push_v(need_vt[r + 1])
            attention_r(hp, r, pop_filler, len(fill))
            if r == 0 and hp == 0:
                # prefetch all remaining q/k token chunks while SP is idle
                while state["x"] < NR:
                    load_x_chunk(state["x"])
                    state["x"] += 1
            if hp == NPAIR - 1:
                for t in range(4 * r, 4 * (r + 1)):
                    for nh in range(2):
                        fill.append((("op", t, nh),
                                     lambda t=t, nh=nh:
                                     o_proj_unit(t, nh, (0, 1, 2, 3), outp)))
    while fill:
        fill.popleft()[1]()


_CACHE = {}
RUN_WALLS = []
LAST_RESULTS = None


def _get_program(mask_key, live, n_pat):
    if mask_key not in _CACHE:
        _CACHE[mask_key] = build_program(live, n_pat)
    return _CACHE[mask_key]


def _hilo(a):
    """Split a float32 array into fp8-e4m3 hi + lo parts."""
    hi = a.astype(E4)
    lo = (a - hi.astype(np.float32)).astype(E4)
    return np.ascontiguousarray(hi), np.ascontiguousarray(lo)


def core_inputs(c, q, k, v, wq, bq, wk, bk, wv, bv, wo, pats):
    """Build the in_map for core c (batch c//2, head-group c%2)."""
    b, g = divmod(c, 2)
    gs = slice(DL * g, DL * (g + 1))
    im = {}

    def chunked(hi, lo):
        # [2, D, n] -> [NC2, 128, 2(a), 2(i), n]
        a = np.stack([hi, lo])
        n = a.shape[-1]
        return np.ascontiguousarray(
            a.reshape(2, NC2, 2, 128, n).transpose(1, 3, 0, 2, 4))

    for nm, x in (("xq", q), ("xk", k), ("xv", v)):
        im[nm + "8"] = chunked(*_hilo(np.ascontiguousarray(x[b].T)))
    for nm, w in (("wq", wq), ("wk", wk), ("wv", wv)):
        im[nm + "8"] = chunked(*_hilo(np.ascontiguousarray(w[gs].T)
                                      * WSCALE))
    # wo stays bf16/unscaled: its row-slice, transposed
    im["wot"] = np.ascontiguousarray(wo[:, gs].T).astype(BF)
    bqk = np.concatenate([bq[gs].reshape(NPAIR, 128).T,
                          bk[gs].reshape(NPAIR, 128).T],
                         axis=1).astype(np.float32) * WSCALE
    im["bqkt"] = np.ascontiguousarray(bqk)
    im["bv"] = (bv[gs].reshape(1, DL) * WSCALE).astype(BF)
    im["pats"] = pats
    return im


def kernel(q, k, v, mask, wq, bq, wk, bk, wv, bv, wo, bo):
    q = np.asarray(q, np.float32)
    k = np.asarray(k, np.float32)
    v = np.asarray(v, np.float32)
    mask = np.asarray(mask, bool)
    wq, wk, wv, wo = (np.asarray(w, np.float32) for w in (wq, wk, wv, wo))
    bq, bk, bv, bo = (np.asarray(b, np.float32) for b in (bq, bk, bv, bo))

    live, patterns = classify_mask(mask)
    n_pat = len(patterns)
    nc = _get_program(mask.tobytes(), live, n_pat)

    pats = np.zeros((max(n_pat, 1), 128, 512), BF)
    for i, p in enumerate(patterns):
        pats[i] = p.astype(BF)

    in_maps = [core_inputs(c, q, k, v, wq, bq, wk, bk, wv, bv, wo, pats)
               for c in range(NCORES)]

    import time as _time
    _t0 = _time.time()
    res = run_bass_kernel_spmd(nc, in_maps, core_ids=list(range(NCORES)))
    RUN_WALLS.append(_time.time() - _t0)
    global LAST_RESULTS
    LAST_RESULTS = res

    out = np.empty((B, S, D), np.float32)
    for b in range(B):
        out[b] = (res.results[2 * b]["outp"] + res.results[2 * b + 1]["outp"]
                  + bo)
    return out
